# revision 3
# baseline (speedup 1.0000x reference)
"""Quantized-weight batched linear: out[b,n,m] = sum_k deq(qweight)[n,k] * x[b,k,m].

Strategy:
  - Host: dequantize weight (fp32, exact same formula as the oracle), transpose
    to (K, N), round weights + activations to bf16.
  - Device (8 cores, data-parallel over batch B=64 -> 8 batches/core):
    PE bf16 matmuls, K accumulated in PSUM over 8 chunks of 128,
    N tiled 8x128 (PSUM partitions), M tiled 2x512 (PSUM bank free-dim).
  - Gather core outputs along batch -> (64, 1024, 1024) fp32.
"""

import numpy as np
import ml_dtypes

N = 1024  # output rows (weight rows)
K = 1024  # reduction dim
M = 1024  # columns of x per batch
NGROUP = 16
GS = K // NGROUP
B = 64
NCORES = 8
BPC = B // NCORES  # batches per core

_CACHE = {}
LAST_RESULT = None  # BassKernelResults of the most recent run (for profiling)


def _build_nc(bpc=BPC, k=K, n=N, m=M):
    import concourse.mybir as mybir
    import concourse.tile as tile
    from concourse import bacc

    kc = k // 128   # contraction chunks (partition dim)
    nt = n // 128   # output-row tiles (PSUM partition dim)
    mt = m // 512   # moving free-dim tiles (one PSUM bank each)

    nc = bacc.Bacc(None, target_bir_lowering=False, debug=False)
    wt = nc.dram_tensor("wt", [k, n], mybir.dt.bfloat16, kind="ExternalInput")
    xs = nc.dram_tensor("xs", [bpc, k, m], mybir.dt.bfloat16, kind="ExternalInput")
    out = nc.dram_tensor("out", [bpc, n, m], mybir.dt.float32, kind="ExternalOutput")

    with tile.TileContext(nc) as tc:
        with (
            tc.tile_pool(name="wpool", bufs=1) as wpool,
            tc.tile_pool(name="xpool", bufs=2 * kc) as xpool,
            tc.tile_pool(name="opool", bufs=8) as opool,
            tc.tile_pool(name="psum", bufs=8, space="PSUM") as psum_pool,
        ):
            # Weights resident in SBUF for the whole kernel: kc slabs of
            # [128 (k-part), n] bf16.
            wsb = []
            for kk in range(kc):
                wtile = wpool.tile([128, n], mybir.dt.bfloat16, tag=f"w{kk}", name=f"w{kk}")
                nc.sync.dma_start(out=wtile[:], in_=wt[kk * 128:(kk + 1) * 128, :])
                wsb.append(wtile)

            for b in range(bpc):
                # Stream one batch of x: kc slabs of [128, m] bf16.
                xtiles = []
                for kk in range(kc):
                    xt = xpool.tile([128, m], mybir.dt.bfloat16, tag="x", name=f"x{b}_{kk}")
                    nc.sync.dma_start(out=xt[:], in_=xs[b, kk * 128:(kk + 1) * 128, :])
                    xtiles.append(xt)

                for n0 in range(nt):
                    ps = [
                        psum_pool.tile([128, 512], mybir.dt.float32, tag="ps", name=f"ps{b}_{n0}_{m0}")
                        for m0 in range(mt)
                    ]
                    for kk in range(kc):
                        lhsT = wsb[kk][:, n0 * 128:(n0 + 1) * 128]
                        for m0 in range(mt):
                            nc.tensor.matmul(
                                ps[m0][:],
                                lhsT,
                                xtiles[kk][:, m0 * 512:(m0 + 1) * 512],
                                start=(kk == 0),
                                stop=(kk == kc - 1),
                            )
                    for m0 in range(mt):
                        ot = opool.tile([128, 512], mybir.dt.float32, tag="o", name=f"o{b}_{n0}_{m0}")
                        # Split PSUM->SBUF drain across DVE and ACT engines.
                        if m0 % 2 == 0:
                            nc.vector.tensor_copy(ot[:], ps[m0][:])
                        else:
                            nc.scalar.copy(ot[:], ps[m0][:])
                        nc.sync.dma_start(
                            out=out[b, n0 * 128:(n0 + 1) * 128, m0 * 512:(m0 + 1) * 512],
                            in_=ot[:],
                        )
    nc.compile()
    return nc


def _dequant_wt(qweight, qrange, qmin):
    # Matches reference: w = q * qrange + qmin per (row, group), fp32.
    q = np.asarray(qweight).astype(np.float32).reshape(N, NGROUP, GS)
    qr = np.asarray(qrange).astype(np.float32).reshape(N, NGROUP, 1)
    qm = np.asarray(qmin).astype(np.float32).reshape(N, NGROUP, 1)
    w = (q * qr + qm).reshape(N, K)
    return np.ascontiguousarray(w.T).astype(ml_dtypes.bfloat16)  # (K, N)


def kernel(x, qweight, qrange, qmin):
    global LAST_RESULT
    from concourse.bass_utils import run_bass_kernel_spmd

    wt_host = _dequant_wt(qweight, qrange, qmin)
    xb = np.asarray(x).astype(ml_dtypes.bfloat16)  # (B, K, M)

    if "nc" not in _CACHE:
        _CACHE["nc"] = _build_nc()
    nc = _CACHE["nc"]

    in_maps = [
        {"wt": wt_host, "xs": np.ascontiguousarray(xb[c * BPC:(c + 1) * BPC])}
        for c in range(NCORES)
    ]
    LAST_RESULT = run_bass_kernel_spmd(nc, in_maps, core_ids=list(range(NCORES)))
    outs = [r["out"] for r in LAST_RESULT.results]
    return np.ascontiguousarray(np.concatenate(outs, axis=0)).astype(np.float32, copy=False)
